# revision 16
# baseline (speedup 1.0000x reference)
"""Trainium2 Bass kernel for the BinaryMechanismSSM problem.

Full inputs in, full outputs out. Internally: batch (128) sharded 8 ways
(16 rows/core). Per core:
  Phase 1 (interleaved with phase 2 as PE filler work): projections
    bx_m = 512*(x @ Wm^T + bm)  (fp16 staging planes, one per (m, j-block)),
    g = sigmoid(x @ G^T + gb)   (fp16 planes per j-block).
    fp16 matmuls with N=512 token tiles; bias add on DVE (bx) / ACT (gate).
  Phase 2: T sequential steps. State st[p, j*16+b] = s[b, 128j+p] held in
    fp16 [128, 64]. Per step: one fp16 identity matmul injects the staged
    512*bx planes into PSUM [128, (m,j,b)], then 32 fp8(e4m3, x512) weight
    matmuls accumulate A_m @ s in k-major rounds; two ACT tanh ops
    (scale=1/512) over psum halves; DVE blend with precomputed gate
    coefficient planes; fp16 state written into a 4-step-wide staging tile
    DMA'd to DRAM every 4 steps. Host re-layouts to [B, T+1, S] f32.
"""
import numpy as np
import ml_dtypes

B_FULL = 128
T_FULL = 1024
I_DIM = 256
S_DIM = 512
N_CORES = 8
B_LOC = B_FULL // N_CORES  # 16

_cache = {}


def _build(alpha: float, z: int, T: int):
    import concourse.bass as bass
    from concourse import bacc
    import concourse.mybir as mybir
    from concourse.tile import TileContext

    dt = mybir.dt
    AF = mybir.ActivationFunctionType
    ALU = mybir.AluOpType

    TOK = T * B_LOC          # tokens per core
    NTT = TOK // 512         # phase-1 token tiles (32 steps each)
    NG = T // 16             # phase-2 step groups
    NREC = 2 if z != 0 else 1
    NMAT = NREC + 1
    W = NREC * 64            # psum width per step
    SC = 512.0               # fp8/bx prescale

    a0 = float(1.0 - alpha) if z != 0 else 1.0
    a1 = float(alpha)
    # blend fold: f = a0*f0 + a1*f1 = gbase^-1... v = f0 + r*f1, u = gbase*g*v
    if z != 0 and a0 >= 1e-6:
        gbase, rfold = a0, a1 / a0
    elif z != 0:
        gbase, rfold = a1, None  # alpha == 1: f = f1 only
    else:
        gbase, rfold = 1.0, None

    nc = bacc.Bacc("TRN2", target_bir_lowering=False, debug=False,
                   num_devices=N_CORES)

    xT_d = nc.declare_dram_parameter("xT", [2, 128, TOK], dt.float16, isOutput=False)
    pw_d = nc.declare_dram_parameter("pw", [128, NMAT * 2 * 4 * 128], dt.float16, isOutput=False)
    bias_d = nc.declare_dram_parameter("bias", [128, 4 * NMAT], dt.float32, isOutput=False)
    aw_d = nc.declare_dram_parameter("aw", [128, NREC * 16 * 128], dt.float8e4, isOutput=False)
    s0_d = nc.declare_dram_parameter("s0T", [128, 64], dt.float16, isOutput=False)
    iden_d = nc.declare_dram_parameter("iden", [128, 128], dt.float16, isOutput=False)
    stg_d = nc.declare_dram_parameter("stg", [128, T * 64], dt.float16, isOutput=True)

    with TileContext(nc) as tc:
      with tc.tile_pool(name="dram", bufs=1, space="DRAM") as dpool:
        bplane = [[dpool.tile([128, TOK], dt.float16, tag=f"bp{m}{j}",
                              name=f"bp{m}{j}") for j in range(4)]
                  for m in range(NREC)]
        gplane = [dpool.tile([128, TOK], dt.float16, tag=f"gp{j}",
                             name=f"gp{j}") for j in range(4)]

        with (
            tc.tile_pool(name="wpool", bufs=1) as wp,
            tc.tile_pool(name="p1x", bufs=3) as p1x,
            tc.tile_pool(name="p1o", bufs=6) as p1o,
            tc.tile_pool(name="p1ps", bufs=3, space="PSUM") as p1ps,
            tc.tile_pool(name="p2in", bufs=3) as p2in,
            tc.tile_pool(name="p2st", bufs=3) as p2st,
            tc.tile_pool(name="p2c", bufs=4) as p2c,
            tc.tile_pool(name="p2ps", bufs=2, space="PSUM") as p2ps,
        ):
            # ---- persistent weights ----
            pw = wp.tile([128, NMAT * 2 * 4 * 128], dt.float16)
            nc.sync.dma_start(pw[:], pw_d[:])
            bias = wp.tile([128, 4 * NMAT], dt.float32)
            nc.sync.dma_start(bias[:], bias_d[:])
            aw = wp.tile([128, NREC * 16 * 128], dt.float8e4)
            nc.sync.dma_start(aw[:], aw_d[:])
            iden = wp.tile([128, 128], dt.float16)
            nc.sync.dma_start(iden[:], iden_d[:])
            s0t = wp.tile([128, 64], dt.float16)
            nc.sync.dma_start(s0t[:], s0_d[:])

            # ---- phase-1 unit emitters ----
            xt_cur = [None]

            def p1_load_x(tt):
                xt = p1x.tile([128, 1024], dt.float16, tag="xt")
                for i in range(2):
                    nc.sync.dma_start(xt[:, i * 512:(i + 1) * 512],
                                      xT_d[i, :, tt * 512:(tt + 1) * 512])
                xt_cur[0] = xt

            def p1_unit(tt, mat, j):
                """One (mat, j) projection unit of token tile tt."""
                xt = xt_cur[0]
                ps = p1ps.tile([128, 512], dt.float32, tag="p1ps")
                for i in range(2):
                    blk = ((mat * 2 + i) * 4 + j) * 128
                    nc.tensor.matmul(ps[:], pw[:, blk:blk + 128],
                                     xt[:, i * 512:(i + 1) * 512],
                                     start=(i == 0), stop=(i == 1))
                if mat == NMAT - 1:  # gate
                    og = p1o.tile([128, 512], dt.float16, tag="og")
                    nc.scalar.activation(og[:], ps[:], AF.Sigmoid,
                                         bias=bias[:, mat * 4 + j:mat * 4 + j + 1])
                    nc.sync.dma_start(gplane[j][:, tt * 512:(tt + 1) * 512], og[:])
                else:
                    ob = p1o.tile([128, 512], dt.float16, tag="ob")
                    nc.vector.tensor_scalar(
                        ob[:], ps[:], bias[:, mat * 4 + j:mat * 4 + j + 1], None,
                        ALU.add)
                    nc.sync.dma_start(bplane[mat][j][:, tt * 512:(tt + 1) * 512],
                                      ob[:])

            p1_units = [(mat, j) for mat in range(NMAT) for j in range(4)]
            NU = len(p1_units)  # 12 units per token tile

            # prologue: first two token tiles fully
            for tt in range(min(2, NTT)):
                p1_load_x(tt)
                for (mat, j) in p1_units:
                    p1_unit(tt, mat, j)

            # phase-1 work queue for interleaving: token tiles 2..NTT-1
            p1q = [(tt, mat, j) for tt in range(2, NTT)
                   for (mat, j) in p1_units]
            # schedule: NG groups; issue len(p1q)/NG units per group spread
            # across its 16 steps
            p1pos = [0]

            def p1_drain(upto):
                while p1pos[0] < min(upto, len(p1q)):
                    tt, mat, j = p1q[p1pos[0]]
                    if (mat, j) == p1_units[0]:
                        p1_load_x(tt)
                    p1_unit(tt, mat, j)
                    p1pos[0] += 1

            # ---- phase 2 ----
            prev = s0t  # tile holding current state
            prev_off = 0  # column offset (x64) within prev tile
            stw = None

            CH = W // 2  # psum-bank / half width

            for g in range(NG):
                # bx staging loaded with DMA-side interleave to layout
                # (t, h, m, j%2, b): per-step per-bank slices are contiguous
                pjbx = p2in.tile([128, 16 * W], dt.float16, tag="pjbx")
                pjbxw = pjbx[:].rearrange(
                    "p (t h m jj b) -> p t h m jj b",
                    t=16, h=2, m=NREC, jj=2, b=16)
                for m in range(NREC):
                    for j in range(4):
                        src = bplane[m][j][:].rearrange(
                            "p (s b) -> p s b", b=16)[:, g * 16:(g + 1) * 16, :]
                        nc.sync.dma_start(pjbxw[:, :, j // 2, m, j % 2, :], src)
                # gate staging interleaved to (t, j, b)
                pjg = p2in.tile([128, 1024], dt.float16, tag="pjg")
                pjgw = pjg[:].rearrange("p (t j b) -> p t j b", t=16, j=4)
                for j in range(4):
                    src = gplane[j][:].rearrange(
                        "p (s b) -> p s b", b=16)[:, g * 16:(g + 1) * 16, :]
                    nc.sync.dma_start(pjgw[:, :, j, :], src)

                # gate coefficient planes, all-contiguous DVE ops (layout
                # (t,j,b)): gco = gbase*g ; g1m = 1-g.  The alpha blend is
                # folded into a single stt op per half (v = ft1*r + ft0),
                # so only one gco plane is needed.
                gco = p2in.tile([128, 1024], dt.float16, tag="gco")
                nc.vector.tensor_scalar_mul(gco[:], pjg[:], gbase)
                g1m = p2in.tile([128, 1024], dt.float16, tag="g1m")
                nc.vector.tensor_scalar(g1m[:], pjg[:], -1.0, 1.0,
                                        ALU.mult, ALU.add)

                # phase-1 filler budget: keep plane writes >= 4 groups ahead
                # of their phase-2 readers (tile X read at group 2X, issued
                # by end of group 2X-4)
                p1_start = min((g // 2) * NU, len(p1q))
                p1_end = min((g // 2 + 1) * NU, len(p1q))

                for tt in range(16):
                    t = g * 16 + tt
                    q = tt % 4
                    if q == 0:
                        stw = p2st.tile([128, 256], dt.float16, tag="stw")
                    if prev is s0t:
                        prevc = prev[:]
                    else:
                        prevc = prev[:, prev_off * 64:(prev_off + 1) * 64]

                    # one psum bank per tanh half: cols (m, j%2, b)
                    psh = [p2ps.tile([128, CH], dt.float32, tag=f"ps{h}",
                                     name=f"ps{h}_{t}")
                           for h in range(2)]

                    def a_mm(j, k, last):
                        for m in range(NREC):
                            blk = ((m * 4 + j) * 4 + k) * 128
                            nc.tensor.matmul(
                                psh[j // 2][:, (m * 2 + j % 2) * 16:
                                            (m * 2 + j % 2 + 1) * 16],
                                aw[:, blk:blk + 128],
                                prevc[:, k * 16:(k + 1) * 16],
                                start=False, stop=last)

                    # Early bank E = half 1 (j23), late bank L = half 0.
                    # Interleaved rounds: stale-chunk consumers (k2,k3)
                    # first, fresh-chunk consumers (k0,k1) last; E's blocks
                    # before L's within each pair so bank E completes first.
                    for h in (1, 0):
                        nc.tensor.matmul(
                            psh[h][:], iden[:],
                            pjbx[:, tt * W + h * CH:tt * W + (h + 1) * CH],
                            start=True, stop=False)
                    for ki, k in enumerate((2, 3, 0, 1)):
                        for j in (2, 3, 0, 1):
                            a_mm(j, k, last=(ki == 3))

                    # m2 = (1-g_t) * s halves, each on its half's engine
                    m2 = p2c.tile([128, 64], dt.float32, tag="m2")
                    nc.gpsimd.tensor_tensor(
                        m2[:, 32:64], prevc[:, 32:64],
                        g1m[:, tt * 64 + 32:tt * 64 + 64], ALU.mult)
                    nc.vector.tensor_tensor(
                        m2[:, 0:32], prevc[:, 0:32],
                        g1m[:, tt * 64:tt * 64 + 32], ALU.mult)

                    # per half (E first): tanh -> fold -> gate -> state chunk
                    # E chain on GpSimd, L chain on DVE (parallel tails)
                    for h, eng in ((1, nc.gpsimd), (0, nc.vector)):
                        ft = p2c.tile([128, NREC * 32], dt.float16, tag=f"ft{h}")
                        nc.scalar.activation(ft[:], psh[h][:], AF.Tanh,
                                             scale=1.0 / SC)
                        gslc = gco[:, tt * 64 + h * 32:tt * 64 + (h + 1) * 32]
                        u2 = p2c.tile([128, 32], dt.float32, tag=f"u{h}")
                        if NREC == 2 and rfold is not None:
                            v2 = p2c.tile([128, 32], dt.float32, tag=f"v{h}")
                            if rfold == 1.0:
                                eng.tensor_tensor(v2[:], ft[:, 32:64],
                                                  ft[:, 0:32], ALU.add)
                            else:
                                # stt is DVE-only (TensorScalarPtr not on Pool)
                                nc.vector.scalar_tensor_tensor(
                                    v2[:], ft[:, 32:64], rfold, ft[:, 0:32],
                                    ALU.mult, ALU.add)
                            eng.tensor_tensor(u2[:], v2[:], gslc, ALU.mult)
                        elif NREC == 2:
                            eng.tensor_tensor(u2[:], ft[:, 32:64], gslc,
                                              ALU.mult)
                        else:
                            eng.tensor_tensor(u2[:], ft[:], gslc, ALU.mult)
                        eng.tensor_tensor(
                            stw[:, q * 64 + h * 32:q * 64 + (h + 1) * 32],
                            u2[:], m2[:, h * 32:(h + 1) * 32], ALU.add)

                    prev, prev_off = stw, q

                    # batched output DMA every 4 steps
                    if q == 3:
                        nc.sync.dma_start(
                            stg_d[:, (t - 3) * 64:(t + 1) * 64], stw[:])

                    # phase-1 filler work between steps
                    p1_drain(p1_start + ((tt + 1) * (p1_end - p1_start)) // 16)

                p1_drain(p1_end)
            p1_drain(len(p1q))

    nc.compile()
    return nc


def _pack_lhsT_blocks(Wm, kdim, mdim, dtype):
    """Wm: [mdim*128, kdim*128]; returns [128, kdim*mdim*128] with block
    (k, j) at cols (k*mdim+j)*128 equal to Wm[j-chunk, k-chunk].T."""
    out = np.zeros((128, kdim * mdim * 128), dtype=np.float32)
    for k in range(kdim):
        for j in range(mdim):
            blk = Wm[j * 128:(j + 1) * 128, k * 128:(k + 1) * 128].T
            out[:, (k * mdim + j) * 128:(k * mdim + j + 1) * 128] = blk
    return np.ascontiguousarray(out.astype(dtype))


def kernel(x_seq, s0, A0_w, B0_w, B0_b, A1_w, B1_w, B1_b, gate_w, gate_b,
           alpha, z, _T=None, _trace=False):
    from concourse.bass_utils import run_bass_kernel_spmd

    T = int(_T or T_FULL)
    alpha_f = float(np.asarray(alpha))
    z_i = int(np.asarray(z))
    SC = 512.0

    key = (alpha_f, z_i, T)
    if key not in _cache:
        _cache[key] = _build(alpha_f, z_i, T)
    nc = _cache[key]

    NREC = 2 if z_i != 0 else 1
    NMAT = NREC + 1

    x_seq = np.asarray(x_seq, dtype=np.float32)
    s0 = np.asarray(s0, dtype=np.float32)

    # ---- replicated weights ----
    if z_i != 0:
        bmats = [np.asarray(B0_w), np.asarray(B1_w)]
        bvecs = [np.asarray(B0_b), np.asarray(B1_b)]
        recs = [np.asarray(A0_w), np.asarray(A1_w)]
    else:
        bmats = [np.asarray(B0_w)]
        bvecs = [np.asarray(B0_b)]
        recs = [np.asarray(A0_w)]

    # phase-1 lhsT blocks: bx mats prescaled by SC, gate unscaled
    pw_parts = [_pack_lhsT_blocks(Wm.astype(np.float32) * SC, 2, 4, np.float16)
                for Wm in bmats]
    pw_parts.append(_pack_lhsT_blocks(np.asarray(gate_w).astype(np.float32),
                                      2, 4, np.float16))
    pw = np.ascontiguousarray(np.concatenate(pw_parts, axis=1))

    bias = np.zeros((128, 4 * NMAT), np.float32)
    for mi, bvec in enumerate(bvecs):
        bias[:, mi * 4:(mi + 1) * 4] = (
            bvec.astype(np.float32).reshape(4, 128).T * SC)
    bias[:, NREC * 4:(NREC + 1) * 4] = (
        np.asarray(gate_b).astype(np.float32).reshape(4, 128).T)

    aw = np.concatenate(
        [_pack_lhsT_blocks(A.astype(np.float32) * SC, 4, 4, np.float32)
         for A in recs], axis=1)
    # (m,j,k) block order: _pack gives (k*4+j); need ((m*4+j)*4+k)*128
    aw = aw.reshape(128, NREC, 4, 4, 128).transpose(0, 1, 3, 2, 4)
    aw = np.ascontiguousarray(aw.reshape(128, -1).astype(ml_dtypes.float8_e4m3))

    IDEN = np.ascontiguousarray(np.eye(128, dtype=np.float16))

    # ---- per-core inputs ----
    in_maps = []
    for c in range(N_CORES):
        bc = c * B_LOC
        xc = x_seq[bc:bc + B_LOC, :T]                       # [16, T, 256]
        xT = np.ascontiguousarray(
            xc.transpose(2, 1, 0).reshape(2, 128, T * B_LOC).astype(np.float16))
        s0c = s0[bc:bc + B_LOC]                             # [16, 512]
        s0T = np.ascontiguousarray(
            s0c.T.reshape(4, 128, B_LOC).transpose(1, 0, 2)
            .reshape(128, 64).astype(np.float16))
        in_maps.append({
            "xT": xT, "pw": pw, "bias": bias, "aw": aw, "s0T": s0T,
            "iden": IDEN,
        })

    res = run_bass_kernel_spmd(nc, in_maps, list(range(N_CORES)), trace=_trace)
    if _trace:
        kernel._last_res = res

    out = np.empty((B_FULL, T + 1, S_DIM), np.float32)
    for c in range(N_CORES):
        bc = c * B_LOC
        stg = np.asarray(res.results[c]["stg"])             # [128, T*64] f16
        out[bc:bc + B_LOC, 0] = s0[bc:bc + B_LOC]
        # stg[p, t*64 + j*16 + b] = s_{t+1}[b, j*128 + p]
        st = stg.reshape(128, T, 4, B_LOC).astype(np.float32)
        out[bc:bc + B_LOC, 1:] = (
            st.transpose(3, 1, 2, 0).reshape(B_LOC, T, S_DIM))
    return out


# revision 17
# speedup vs baseline: 1.0279x; 1.0279x over previous
"""Trainium2 Bass kernel for the BinaryMechanismSSM problem.

Full inputs in, full outputs out. Internally: batch (128) sharded 8 ways
(16 rows/core). Per core:
  Phase 1 (interleaved with phase 2 as PE filler work): projections
    bx_m = 512*(x @ Wm^T + bm)  (fp16 staging planes, one per (m, j-block)),
    g = sigmoid(x @ G^T + gb)   (fp16 planes per j-block).
    fp16 matmuls with N=512 token tiles; bias add on DVE (bx) / ACT (gate).
  Phase 2: T sequential steps. State st[p, j*16+b] = s[b, 128j+p] held in
    fp16 [128, 64]. Per step: one fp16 identity matmul injects the staged
    512*bx planes into PSUM [128, (m,j,b)], then 32 fp8(e4m3, x512) weight
    matmuls accumulate A_m @ s in k-major rounds; two ACT tanh ops
    (scale=1/512) over psum halves; DVE blend with precomputed gate
    coefficient planes; fp16 state written into a 4-step-wide staging tile
    DMA'd to DRAM every 4 steps. Host re-layouts to [B, T+1, S] f32.
"""
import numpy as np
import ml_dtypes

B_FULL = 128
T_FULL = 1024
I_DIM = 256
S_DIM = 512
N_CORES = 8
B_LOC = B_FULL // N_CORES  # 16

_cache = {}


def _build(alpha: float, z: int, T: int):
    import concourse.bass as bass
    from concourse import bacc
    import concourse.mybir as mybir
    from concourse.tile import TileContext

    dt = mybir.dt
    AF = mybir.ActivationFunctionType
    ALU = mybir.AluOpType

    TOK = T * B_LOC          # tokens per core
    NTT = TOK // 512         # phase-1 token tiles (32 steps each)
    NG = T // 16             # phase-2 step groups
    NREC = 2 if z != 0 else 1
    NMAT = NREC + 1
    W = NREC * 64            # psum width per step
    SC = 512.0               # fp8/bx prescale

    a0 = float(1.0 - alpha) if z != 0 else 1.0
    a1 = float(alpha)
    # blend fold: f = a0*f0 + a1*f1 = gbase^-1... v = f0 + r*f1, u = gbase*g*v
    if z != 0 and a0 >= 1e-6:
        gbase, rfold = a0, a1 / a0
    elif z != 0:
        gbase, rfold = a1, None  # alpha == 1: f = f1 only
    else:
        gbase, rfold = 1.0, None

    nc = bacc.Bacc("TRN2", target_bir_lowering=False, debug=False,
                   num_devices=N_CORES)

    xT_d = nc.declare_dram_parameter("xT", [2, 128, TOK], dt.float16, isOutput=False)
    pw_d = nc.declare_dram_parameter("pw", [128, NMAT * 2 * 4 * 128], dt.float16, isOutput=False)
    bias_d = nc.declare_dram_parameter("bias", [128, 4 * NMAT], dt.float32, isOutput=False)
    aw_d = nc.declare_dram_parameter("aw", [128, NREC * 16 * 128], dt.float8e4, isOutput=False)
    s0_d = nc.declare_dram_parameter("s0T", [128, 64], dt.float16, isOutput=False)
    iden_d = nc.declare_dram_parameter("iden", [128, 128], dt.float16, isOutput=False)
    stg_d = nc.declare_dram_parameter("stg", [128, T * 64], dt.float16, isOutput=True)

    with TileContext(nc) as tc:
      with tc.tile_pool(name="dram", bufs=1, space="DRAM") as dpool:
        bplane = [[dpool.tile([128, TOK], dt.float16, tag=f"bp{m}{j}",
                              name=f"bp{m}{j}") for j in range(4)]
                  for m in range(NREC)]
        gplane = [dpool.tile([128, TOK], dt.float16, tag=f"gp{j}",
                             name=f"gp{j}") for j in range(4)]

        with (
            tc.tile_pool(name="wpool", bufs=1) as wp,
            tc.tile_pool(name="p1x", bufs=3) as p1x,
            tc.tile_pool(name="p1o", bufs=6) as p1o,
            tc.tile_pool(name="p1ps", bufs=3, space="PSUM") as p1ps,
            tc.tile_pool(name="p2in", bufs=3) as p2in,
            tc.tile_pool(name="p2st", bufs=3) as p2st,
            tc.tile_pool(name="p2c", bufs=4) as p2c,
            tc.tile_pool(name="p2ps", bufs=2, space="PSUM") as p2ps,
        ):
            # ---- persistent weights ----
            pw = wp.tile([128, NMAT * 2 * 4 * 128], dt.float16)
            nc.sync.dma_start(pw[:], pw_d[:])
            bias = wp.tile([128, 4 * NMAT], dt.float32)
            nc.sync.dma_start(bias[:], bias_d[:])
            aw = wp.tile([128, NREC * 16 * 128], dt.float8e4)
            nc.sync.dma_start(aw[:], aw_d[:])
            iden = wp.tile([128, 128], dt.float16)
            nc.sync.dma_start(iden[:], iden_d[:])
            s0t = wp.tile([128, 64], dt.float16)
            nc.sync.dma_start(s0t[:], s0_d[:])

            # ---- phase-1 unit emitters ----
            xt_cur = [None]

            def p1_load_x(tt):
                xt = p1x.tile([128, 1024], dt.float16, tag="xt")
                for i in range(2):
                    nc.sync.dma_start(xt[:, i * 512:(i + 1) * 512],
                                      xT_d[i, :, tt * 512:(tt + 1) * 512])
                xt_cur[0] = xt

            def p1_unit(tt, mat, j):
                """One (mat, j) projection unit of token tile tt."""
                xt = xt_cur[0]
                ps = p1ps.tile([128, 512], dt.float32, tag="p1ps")
                for i in range(2):
                    blk = ((mat * 2 + i) * 4 + j) * 128
                    nc.tensor.matmul(ps[:], pw[:, blk:blk + 128],
                                     xt[:, i * 512:(i + 1) * 512],
                                     start=(i == 0), stop=(i == 1))
                if mat == NMAT - 1:  # gate
                    og = p1o.tile([128, 512], dt.float16, tag="og")
                    nc.scalar.activation(og[:], ps[:], AF.Sigmoid,
                                         bias=bias[:, mat * 4 + j:mat * 4 + j + 1])
                    nc.sync.dma_start(gplane[j][:, tt * 512:(tt + 1) * 512], og[:])
                else:
                    ob = p1o.tile([128, 512], dt.float16, tag="ob")
                    nc.vector.tensor_scalar(
                        ob[:], ps[:], bias[:, mat * 4 + j:mat * 4 + j + 1], None,
                        ALU.add)
                    nc.sync.dma_start(bplane[mat][j][:, tt * 512:(tt + 1) * 512],
                                      ob[:])

            p1_units = [(mat, j) for mat in range(NMAT) for j in range(4)]
            NU = len(p1_units)  # 12 units per token tile

            # prologue: first two token tiles fully
            for tt in range(min(2, NTT)):
                p1_load_x(tt)
                for (mat, j) in p1_units:
                    p1_unit(tt, mat, j)

            # phase-1 work queue for interleaving: token tiles 2..NTT-1
            p1q = [(tt, mat, j) for tt in range(2, NTT)
                   for (mat, j) in p1_units]
            # schedule: NG groups; issue len(p1q)/NG units per group spread
            # across its 16 steps
            p1pos = [0]

            def p1_drain(upto):
                while p1pos[0] < min(upto, len(p1q)):
                    tt, mat, j = p1q[p1pos[0]]
                    if (mat, j) == p1_units[0]:
                        p1_load_x(tt)
                    p1_unit(tt, mat, j)
                    p1pos[0] += 1

            # ---- phase 2 ----
            prev = s0t  # tile holding current state
            prev_off = 0  # column offset (x64) within prev tile
            stw = None

            CH = W // 2  # psum-bank / half width

            for g in range(NG):
                # bx staging loaded with DMA-side interleave to layout
                # (t, h, m, j%2, b): per-step per-bank slices are contiguous
                pjbx = p2in.tile([128, 16 * W], dt.float16, tag="pjbx")
                pjbxw = pjbx[:].rearrange(
                    "p (t h m jj b) -> p t h m jj b",
                    t=16, h=2, m=NREC, jj=2, b=16)
                for m in range(NREC):
                    for j in range(4):
                        src = bplane[m][j][:].rearrange(
                            "p (s b) -> p s b", b=16)[:, g * 16:(g + 1) * 16, :]
                        nc.sync.dma_start(pjbxw[:, :, j // 2, m, j % 2, :], src)
                # gate staging interleaved to (t, j, b)
                pjg = p2in.tile([128, 1024], dt.float16, tag="pjg")
                pjgw = pjg[:].rearrange("p (t j b) -> p t j b", t=16, j=4)
                for j in range(4):
                    src = gplane[j][:].rearrange(
                        "p (s b) -> p s b", b=16)[:, g * 16:(g + 1) * 16, :]
                    nc.sync.dma_start(pjgw[:, :, j, :], src)

                # gate coefficient planes, all-contiguous DVE ops (layout
                # (t,j,b)): gco = gbase*g ; g1m = 1-g.  The alpha blend is
                # folded into a single stt op per half (v = ft1*r + ft0),
                # so only one gco plane is needed.
                gco = p2in.tile([128, 1024], dt.float16, tag="gco")
                nc.vector.tensor_scalar_mul(gco[:], pjg[:], gbase)
                g1m = p2in.tile([128, 1024], dt.float16, tag="g1m")
                nc.vector.tensor_scalar(g1m[:], pjg[:], -1.0, 1.0,
                                        ALU.mult, ALU.add)

                # phase-1 filler budget: keep plane writes >= 4 groups ahead
                # of their phase-2 readers (tile X read at group 2X, issued
                # by end of group 2X-4)
                p1_start = min((g // 2) * NU, len(p1q))
                p1_end = min((g // 2 + 1) * NU, len(p1q))

                for tt in range(16):
                    t = g * 16 + tt
                    q = tt % 4
                    if q == 0:
                        stw = p2st.tile([128, 256], dt.float16, tag="stw")
                    if prev is s0t:
                        prevc = prev[:]
                    else:
                        prevc = prev[:, prev_off * 64:(prev_off + 1) * 64]

                    # one psum bank per tanh half: cols (m, j%2, b)
                    psh = [p2ps.tile([128, CH], dt.float32, tag=f"ps{h}",
                                     name=f"ps{h}_{t}")
                           for h in range(2)]

                    def a_mm(j, k, last):
                        for m in range(NREC):
                            blk = ((m * 4 + j) * 4 + k) * 128
                            nc.tensor.matmul(
                                psh[j // 2][:, (m * 2 + j % 2) * 16:
                                            (m * 2 + j % 2 + 1) * 16],
                                aw[:, blk:blk + 128],
                                prevc[:, k * 16:(k + 1) * 16],
                                start=False, stop=last)

                    # Early bank E = half 1 (j23), late bank L = half 0.
                    # Interleaved rounds: stale-chunk consumers (k2,k3)
                    # first, fresh-chunk consumers (k0,k1) last; E's blocks
                    # before L's within each pair so bank E completes first.
                    for h in (1, 0):
                        nc.tensor.matmul(
                            psh[h][:], iden[:],
                            pjbx[:, tt * W + h * CH:tt * W + (h + 1) * CH],
                            start=True, stop=False)
                    for h in (1, 0):
                        for ki, k in enumerate((2, 3, 0, 1)):
                            for j in (2 * h, 2 * h + 1):
                                a_mm(j, k, last=(ki == 3))

                    # m2 = (1-g_t) * s halves, each on its half's engine
                    m2 = p2c.tile([128, 64], dt.float32, tag="m2")
                    nc.gpsimd.tensor_tensor(
                        m2[:, 32:64], prevc[:, 32:64],
                        g1m[:, tt * 64 + 32:tt * 64 + 64], ALU.mult)
                    nc.vector.tensor_tensor(
                        m2[:, 0:32], prevc[:, 0:32],
                        g1m[:, tt * 64:tt * 64 + 32], ALU.mult)

                    # per half (E first): tanh -> fold -> gate -> state chunk
                    # E chain on GpSimd, L chain on DVE (parallel tails)
                    for h, eng in ((1, nc.gpsimd), (0, nc.vector)):
                        ft = p2c.tile([128, NREC * 32], dt.float16, tag=f"ft{h}")
                        nc.scalar.activation(ft[:], psh[h][:], AF.Tanh,
                                             scale=1.0 / SC)
                        gslc = gco[:, tt * 64 + h * 32:tt * 64 + (h + 1) * 32]
                        u2 = p2c.tile([128, 32], dt.float32, tag=f"u{h}")
                        if NREC == 2 and rfold is not None:
                            v2 = p2c.tile([128, 32], dt.float32, tag=f"v{h}")
                            if rfold == 1.0:
                                eng.tensor_tensor(v2[:], ft[:, 32:64],
                                                  ft[:, 0:32], ALU.add)
                            else:
                                # stt is DVE-only (TensorScalarPtr not on Pool)
                                nc.vector.scalar_tensor_tensor(
                                    v2[:], ft[:, 32:64], rfold, ft[:, 0:32],
                                    ALU.mult, ALU.add)
                            eng.tensor_tensor(u2[:], v2[:], gslc, ALU.mult)
                        elif NREC == 2:
                            eng.tensor_tensor(u2[:], ft[:, 32:64], gslc,
                                              ALU.mult)
                        else:
                            eng.tensor_tensor(u2[:], ft[:], gslc, ALU.mult)
                        eng.tensor_tensor(
                            stw[:, q * 64 + h * 32:q * 64 + (h + 1) * 32],
                            u2[:], m2[:, h * 32:(h + 1) * 32], ALU.add)

                    prev, prev_off = stw, q

                    # batched output DMA every 4 steps
                    if q == 3:
                        nc.sync.dma_start(
                            stg_d[:, (t - 3) * 64:(t + 1) * 64], stw[:])

                    # phase-1 filler work between steps
                    p1_drain(p1_start + ((tt + 1) * (p1_end - p1_start)) // 16)

                p1_drain(p1_end)
            p1_drain(len(p1q))

    nc.compile()
    return nc


def _pack_lhsT_blocks(Wm, kdim, mdim, dtype):
    """Wm: [mdim*128, kdim*128]; returns [128, kdim*mdim*128] with block
    (k, j) at cols (k*mdim+j)*128 equal to Wm[j-chunk, k-chunk].T."""
    out = np.zeros((128, kdim * mdim * 128), dtype=np.float32)
    for k in range(kdim):
        for j in range(mdim):
            blk = Wm[j * 128:(j + 1) * 128, k * 128:(k + 1) * 128].T
            out[:, (k * mdim + j) * 128:(k * mdim + j + 1) * 128] = blk
    return np.ascontiguousarray(out.astype(dtype))


def kernel(x_seq, s0, A0_w, B0_w, B0_b, A1_w, B1_w, B1_b, gate_w, gate_b,
           alpha, z, _T=None, _trace=False):
    from concourse.bass_utils import run_bass_kernel_spmd

    T = int(_T or T_FULL)
    alpha_f = float(np.asarray(alpha))
    z_i = int(np.asarray(z))
    SC = 512.0

    key = (alpha_f, z_i, T)
    if key not in _cache:
        _cache[key] = _build(alpha_f, z_i, T)
    nc = _cache[key]

    NREC = 2 if z_i != 0 else 1
    NMAT = NREC + 1

    x_seq = np.asarray(x_seq, dtype=np.float32)
    s0 = np.asarray(s0, dtype=np.float32)

    # ---- replicated weights ----
    if z_i != 0:
        bmats = [np.asarray(B0_w), np.asarray(B1_w)]
        bvecs = [np.asarray(B0_b), np.asarray(B1_b)]
        recs = [np.asarray(A0_w), np.asarray(A1_w)]
    else:
        bmats = [np.asarray(B0_w)]
        bvecs = [np.asarray(B0_b)]
        recs = [np.asarray(A0_w)]

    # phase-1 lhsT blocks: bx mats prescaled by SC, gate unscaled
    pw_parts = [_pack_lhsT_blocks(Wm.astype(np.float32) * SC, 2, 4, np.float16)
                for Wm in bmats]
    pw_parts.append(_pack_lhsT_blocks(np.asarray(gate_w).astype(np.float32),
                                      2, 4, np.float16))
    pw = np.ascontiguousarray(np.concatenate(pw_parts, axis=1))

    bias = np.zeros((128, 4 * NMAT), np.float32)
    for mi, bvec in enumerate(bvecs):
        bias[:, mi * 4:(mi + 1) * 4] = (
            bvec.astype(np.float32).reshape(4, 128).T * SC)
    bias[:, NREC * 4:(NREC + 1) * 4] = (
        np.asarray(gate_b).astype(np.float32).reshape(4, 128).T)

    aw = np.concatenate(
        [_pack_lhsT_blocks(A.astype(np.float32) * SC, 4, 4, np.float32)
         for A in recs], axis=1)
    # (m,j,k) block order: _pack gives (k*4+j); need ((m*4+j)*4+k)*128
    aw = aw.reshape(128, NREC, 4, 4, 128).transpose(0, 1, 3, 2, 4)
    aw = np.ascontiguousarray(aw.reshape(128, -1).astype(ml_dtypes.float8_e4m3))

    IDEN = np.ascontiguousarray(np.eye(128, dtype=np.float16))

    # ---- per-core inputs ----
    in_maps = []
    for c in range(N_CORES):
        bc = c * B_LOC
        xc = x_seq[bc:bc + B_LOC, :T]                       # [16, T, 256]
        xT = np.ascontiguousarray(
            xc.transpose(2, 1, 0).reshape(2, 128, T * B_LOC).astype(np.float16))
        s0c = s0[bc:bc + B_LOC]                             # [16, 512]
        s0T = np.ascontiguousarray(
            s0c.T.reshape(4, 128, B_LOC).transpose(1, 0, 2)
            .reshape(128, 64).astype(np.float16))
        in_maps.append({
            "xT": xT, "pw": pw, "bias": bias, "aw": aw, "s0T": s0T,
            "iden": IDEN,
        })

    res = run_bass_kernel_spmd(nc, in_maps, list(range(N_CORES)), trace=_trace)
    if _trace:
        kernel._last_res = res

    out = np.empty((B_FULL, T + 1, S_DIM), np.float32)
    for c in range(N_CORES):
        bc = c * B_LOC
        stg = np.asarray(res.results[c]["stg"])             # [128, T*64] f16
        out[bc:bc + B_LOC, 0] = s0[bc:bc + B_LOC]
        # stg[p, t*64 + j*16 + b] = s_{t+1}[b, j*128 + p]
        st = stg.reshape(128, T, 4, B_LOC).astype(np.float32)
        out[bc:bc + B_LOC, 1:] = (
            st.transpose(3, 1, 2, 0).reshape(B_LOC, T, S_DIM))
    return out
